# revision 10
# baseline (speedup 1.0000x reference)
"""Trainium2 Bass kernel for nn_CrossAttentionNodesBlockv2.

Cross-attention transformer block (torch nn.MultiheadAttention semantics):
  B=32, LT=256, LP=192, D=512, H=8 heads, head_dim=64, FFN hidden 1024.

Sharding: pure data-parallel over batch, 4 batches per NeuronCore, 8 cores,
no collectives. Each core runs the full block for its batch shard; the host
concatenates shards.

In-kernel layout strategy (per batch):
  - Activations are kept "transposed" (feature-on-partition) for every GEMM's
    operands, so no fp32 DMA transposes are needed; all transposes are PE
    identity-matmul transposes.
  - Attention key-padding masks are folded in as K=1 outer-product matmuls
    accumulating into the scores PSUM (additive -1e4 mask -> exp underflows
    to exactly 0, matching softmax-with--inf).
  - 1/sqrt(hd) is folded into Wq/bq on the host (exact *0.125); the /H of the
    returned head-averaged attention is folded into the softmax reciprocal and
    compensated by Wo*8 (exact) on the value path.
  - Biases (bv/bo/b1/b2) are applied as K=1 ones-outer-product matmuls; bq/bk
    as per-partition tensor-scalar adds on the PSUM->SBUF copy.
  - LayerNorm uses E[x^2]-E[x]^2 with row sums produced for free by the fused
    residual-add (tensor_tensor_reduce) and Square-with-accum on ACT.
  - Output zero-masking is folded into the LN2 rstd (per-partition scalar).
"""

import sys

for _p in ("/opt/trn_rl_repo",):
    if _p not in sys.path:
        sys.path.insert(0, _p)

from contextlib import ExitStack

import numpy as np

import concourse.bass as bass
import concourse.tile as tile
from concourse import bacc, mybir
from concourse.bass_utils import run_bass_kernel_spmd
from concourse.masks import make_identity

F32 = mybir.dt.float32
ALU = mybir.AluOpType
ACTF = mybir.ActivationFunctionType
AX = mybir.AxisListType

B, LT, LP, D, H, HD = 32, 256, 192, 512, 8, 64
DFF = 2 * D
NCORES = 8
BPC = B // NCORES  # batches per core
LN_EPS = 1e-5
NEG = -10000.0
P = 128
NFC = D // P  # 4 feature chunks
NHC = DFF // P  # 8 hidden chunks


def _chunks(n):
    out, o = [], 0
    while o < n:
        out.append((o, min(P, n - o)))
        o += P
    return out


def _build(ln_trivial: bool, bias_trivial: bool):
    nc = bacc.Bacc("TRN2", target_bir_lowering=False, debug=False)

    def din(name, shape):
        return nc.dram_tensor(name, list(shape), F32, kind="ExternalInput").ap()

    def dout(name, shape):
        return nc.dram_tensor(name, list(shape), F32, kind="ExternalOutput").ap()

    tcr_x = din("tcr_x", (BPC, LT, D))
    pmhc_x = din("pmhc_x", (BPC, LP, D))
    mask_tp = din("mask_tp", (BPC, LP))
    mask_pt = din("mask_pt", (BPC, LT))
    keep_tcr = din("keep_tcr", (BPC, LT))
    keep_pep = din("keep_pep", (BPC, LP))

    wd = {}
    for s in ("tp", "pt"):
        for w in ("wq", "wk", "wv", "wo"):
            wd[f"{s}_{w}"] = din(f"{s}_{w}", (D, D))
        for bn in ("bq", "bk"):
            wd[f"{s}_{bn}"] = din(f"{s}_{bn}", (D,))
        if not bias_trivial:
            for bn in ("bv", "bo"):
                wd[f"{s}_{bn}"] = din(f"{s}_{bn}", (D,))
    for s in ("tcr", "pep"):
        wd[f"{s}_w1"] = din(f"{s}_w1", (D, DFF))
        wd[f"{s}_w2"] = din(f"{s}_w2", (DFF, D))
        if not bias_trivial:
            wd[f"{s}_b1"] = din(f"{s}_b1", (DFF,))
            wd[f"{s}_b2"] = din(f"{s}_b2", (D,))
        if not ln_trivial:
            for i in (1, 2):
                wd[f"{s}_g{i}r"] = din(f"{s}_g{i}r", (P, D))
                wd[f"{s}_b{i}r"] = din(f"{s}_b{i}r", (P, D))

    tcr_out = dout("tcr_out", (BPC, LT, D))
    pep_out = dout("pep_out", (BPC, LP, D))
    attn_tp = dout("attn_tp", (BPC, LT, LP))
    attn_pt = dout("attn_pt", (BPC, LP, LT))

    with tile.TileContext(nc) as tc, ExitStack() as ctx:
        consts = ctx.enter_context(tc.tile_pool(name="consts", bufs=1))
        wpool = ctx.enter_context(tc.tile_pool(name="wpool", bufs=1))
        xpool = ctx.enter_context(tc.tile_pool(name="xpool", bufs=1))
        apool = ctx.enter_context(tc.tile_pool(name="apool", bufs=1))
        epool = ctx.enter_context(tc.tile_pool(name="epool", bufs=1))
        ypool = ctx.enter_context(tc.tile_pool(name="ypool", bufs=2))
        small = ctx.enter_context(tc.tile_pool(name="small", bufs=4))
        mpool = ctx.enter_context(tc.tile_pool(name="mpool", bufs=2))
        # PSUM budget (static, 8 banks): psp*2 + sc*2 + tr*2 + ot*2
        ps_proj = ctx.enter_context(tc.tile_pool(name="ps_proj", bufs=2, space="PSUM"))
        ps_sc = ctx.enter_context(tc.tile_pool(name="ps_sc", bufs=2, space="PSUM"))
        ps_tr = ctx.enter_context(tc.tile_pool(name="ps_tr", bufs=2, space="PSUM"))
        ps_ot = ctx.enter_context(tc.tile_pool(name="ps_ot", bufs=2, space="PSUM"))

        ident = consts.tile([P, P], F32, tag="ident")
        make_identity(nc, ident[:])
        # matmul operand base partitions must be in {0, 32, 64}; ones rows at
        # all three bases so any bias row's base has a matching ones row
        ones65 = consts.tile([65, 256], F32, tag="ones65")
        nc.vector.memset(ones65[:], 1.0)
        eps_col = consts.tile([P, 1], F32, tag="eps_col")
        nc.vector.memset(eps_col[:], LN_EPS)

        # bias rows packed at base partitions 0/32/64: name -> (part, col, len)
        row_ix = {
            "tp_bv": (0, 0, D), "tp_bo": (0, 512, D),
            "tcr_b2": (0, 1024, D), "pep_b2": (0, 1536, D),
            "pt_bv": (32, 0, D), "pt_bo": (32, 512, D),
            "tcr_b1": (32, 1024, DFF), "pep_b1": (64, 0, DFF),
        }
        if not bias_trivial:
            rows3 = consts.tile([65, 2048], F32, tag="rows3")
            for nm, (rp, rc, rn) in row_ix.items():
                nc.sync.dma_start(rows3[rp : rp + 1, rc : rc + rn], wd[nm][None, :])

        def bias_row_mm(ps_ap, nm, n_ones, colsl=None):
            """ps += ones[:n_ones]^T (x) bias_row  (K=1 outer product)."""
            if bias_trivial:
                return False
            rp, rc, rn = row_ix[nm]
            c0, c1 = (0, rn) if colsl is None else colsl
            nc.tensor.matmul(
                ps_ap,
                ones65[rp : rp + 1, :n_ones],
                rows3[rp : rp + 1, rc + c0 : rc + c1],
                start=False,
                stop=True,
            )
            return True

        def bias_row_mm_T(ps_ap, nm, n_ones, colsl):
            """ps += bias_row_slice^T (x) ones[:n_ones] (bias on partitions)."""
            if bias_trivial:
                return False
            rp, rc, rn = row_ix[nm]
            c0, c1 = colsl
            nc.tensor.matmul(
                ps_ap,
                rows3[rp : rp + 1, rc + c0 : rc + c1],
                ones65[rp : rp + 1, :n_ones],
                start=False,
                stop=True,
            )
            return True

        def load_w(name, rows_, cols):
            t = wpool.tile([P, rows_ // P, cols], F32, tag=name)
            nc.sync.dma_start(t[:], wd[name].rearrange("(c p) o -> p c o", p=P))
            return t

        def load_col(name, n):
            t = wpool.tile([P, n // P], F32, tag=name)
            nc.sync.dma_start(t[:], wd[name].rearrange("(c p) -> p c", p=P))
            return t

        W = {}
        for s in ("tp", "pt"):
            for w in ("wq", "wk", "wv", "wo"):
                W[f"{s}_{w}"] = load_w(f"{s}_{w}", D, D)
            W[f"{s}_bq"] = load_col(f"{s}_bq", D)
            W[f"{s}_bk"] = load_col(f"{s}_bk", D)
        for s in ("tcr", "pep"):
            W[f"{s}_w1"] = load_w(f"{s}_w1", D, DFF)
            W[f"{s}_w2"] = load_w(f"{s}_w2", DFF, D)
            if not ln_trivial:
                for nm in (f"{s}_g1r", f"{s}_b1r", f"{s}_g2r", f"{s}_b2r"):
                    t = wpool.tile([P, D], F32, tag=nm)
                    nc.sync.dma_start(t[:], wd[nm][:, :])
                    W[nm] = t

        def ln_apply(x_ap, out_ap, qs, gname, bname, keep_sl):
            """out = LN(x) (optionally *g+b), rows scaled by keep (or None)."""
            stats = small.tile([P, 6], F32, tag="bnst")
            nc.vector.bn_stats(stats[:qs, :], x_ap)
            mv = small.tile([P, 2], F32, tag="mv")
            nc.vector.bn_aggr(mv[:qs, :], stats[:qs, :])
            mean = mv[:qs, 0:1]
            sd = small.tile([P, 1], F32, tag="sd")
            nc.scalar.activation(sd[:qs], mv[:qs, 1:2], ACTF.Sqrt, bias=eps_col[:qs, :])
            rstd = small.tile([P, 1], F32, tag="rstd")
            nc.vector.reciprocal(rstd[:qs], sd[:qs])
            if ln_trivial:
                if keep_sl is not None:
                    nc.vector.tensor_tensor(rstd[:qs], rstd[:qs], keep_sl, ALU.mult)
                nc.vector.tensor_scalar(
                    out_ap, x_ap, mean, rstd[:qs], ALU.subtract, ALU.mult
                )
            else:
                nc.vector.tensor_scalar(
                    out_ap, x_ap, mean, rstd[:qs], ALU.subtract, ALU.mult
                )
                nc.vector.tensor_tensor(out_ap, out_ap, W[gname][:qs, :], ALU.mult)
                nc.vector.tensor_tensor(out_ap, out_ap, W[bname][:qs, :], ALU.add)
                if keep_sl is not None:
                    nc.vector.tensor_scalar_mul(out_ap, out_ap, keep_sl)

        def side(pre, ffn, q_x, q_xT, kv_xT, q_len, kv_len, mask_row, mask_part,
                 keep_c, out_d, attn_d, b):
            """One cross-attention + FFN branch for batch b."""
            qcs_l = _chunks(q_len)
            kcs_l = _chunks(kv_len)
            nkc = len(kcs_l)
            wq, wk, wv, wo = (W[f"{pre}_{w}"] for w in ("wq", "wk", "wv", "wo"))
            bqc, bkc = W[f"{pre}_bq"], W[f"{pre}_bk"]
            w1, w2 = W[f"{ffn}_w1"], W[f"{ffn}_w2"]

            # ---- Q^T / K^T projections (feature-on-partition) ----
            QT = apool.tile([P, NFC, 256], F32, tag="QT")
            for ofc in range(NFC):
                ps = ps_proj.tile([P, 512], F32, tag="psp")
                for ifc in range(NFC):
                    nc.tensor.matmul(
                        ps[:, :q_len],
                        wq[:, ifc, ofc * P : (ofc + 1) * P],
                        q_xT[:, ifc, :],
                        start=(ifc == 0),
                        stop=(ifc == NFC - 1),
                    )
                nc.vector.tensor_scalar(
                    QT[:, ofc, :q_len], ps[:, :q_len],
                    bqc[:, ofc : ofc + 1], None, ALU.add,
                )
            KT = apool.tile([P, NFC, 256], F32, tag="KT")
            for ofc in range(NFC):
                ps = ps_proj.tile([P, 512], F32, tag="psp")
                for ifc in range(NFC):
                    nc.tensor.matmul(
                        ps[:, :kv_len],
                        wk[:, ifc, ofc * P : (ofc + 1) * P],
                        kv_xT[:, ifc, :],
                        start=(ifc == 0),
                        stop=(ifc == NFC - 1),
                    )
                nc.vector.tensor_scalar(
                    KT[:, ofc, :kv_len], ps[:, :kv_len],
                    bkc[:, ofc : ofc + 1], None, ALU.add,
                )

            # ---- V (token-on-partition); HT shares the slot later ----
            V = apool.tile([P, 2, D], F32, tag="vht")
            for kc, (ko, ks) in enumerate(kcs_l):
                ps = ps_proj.tile([P, 512], F32, tag="psp")
                for ifc in range(NFC):
                    nc.tensor.matmul(
                        ps[:ks, :],
                        kv_xT[:, ifc, ko : ko + ks],
                        wv[:, ifc, :],
                        start=(ifc == 0),
                        stop=(bias_trivial and ifc == NFC - 1),
                    )
                if not bias_row_mm(ps[:ks, :], f"{pre}_bv", ks):
                    pass
                nc.any.tensor_copy(V[:ks, kc, :], ps[:ks, :])

            # ---- scores -> masked exp -> softmax -> A, A^T -> O^T ----
            OT = apool.tile([P, NFC, 256], F32, tag="KT")
            for qc, (qo, qs) in enumerate(qcs_l):
                E = epool.tile([P, H, 256], F32, tag="E")
                for pr in range(4):
                    ps = ps_sc.tile([P, 2, 256], F32, tag="ps_sc")
                    for j in range(2):
                        h = 2 * pr + j
                        po = (h % 2) * HD
                        fc = h // 2
                        nc.tensor.matmul(
                            ps[:qs, j, :kv_len],
                            QT[po : po + HD, fc, qo : qo + qs],
                            KT[po : po + HD, fc, :kv_len],
                            start=True,
                            stop=False,
                        )
                        nc.tensor.matmul(
                            ps[:qs, j, :kv_len],
                            ones65[0:1, :qs],
                            mask_row,
                            start=False,
                            stop=True,
                        )
                    nc.scalar.activation(
                        E[:qs, 2 * pr : 2 * pr + 2, :kv_len],
                        ps[:qs, :, :kv_len],
                        ACTF.Exp,
                    )
                d = small.tile([P, H], F32, tag="d")
                nc.vector.reduce_sum(d[:qs, :], E[:qs, :, :kv_len], axis=AX.X)
                r8 = small.tile([P, H], F32, tag="r8")
                nc.vector.reciprocal(r8[:qs, :], d[:qs, :])
                nc.vector.tensor_scalar_mul(r8[:qs, :], r8[:qs, :], 1.0 / H)
                nc.vector.tensor_tensor(
                    E[:qs, :, :kv_len],
                    E[:qs, :, :kv_len],
                    r8[:qs, :, None].to_broadcast((qs, H, kv_len)),
                    ALU.mult,
                )

                # transpose normalized attention per head: AT[k, h, kc, q]
                AT = epool.tile([P, H, 2, P], F32, tag="AT")
                for h in range(H):
                    tp = ps_tr.tile([P, 2, P], F32, tag="ps_tr")
                    for kc, (ko, ks) in enumerate(kcs_l):
                        nc.tensor.transpose(
                            tp[:ks, kc, :qs],
                            E[:qs, h, ko : ko + ks],
                            ident[:qs, :qs],
                        )
                        nc.any.tensor_copy(AT[:ks, h, kc, :qs], tp[:ks, kc, :qs])

                # AV matmuls: O^T[dv, q] for this q-chunk (2 heads per bank)
                for f in range(4):
                    pso = ps_ot.tile([P, P], F32, tag="ps_ot")
                    for j in range(2):
                        h = 2 * f + j
                        for kc, (ko, ks) in enumerate(kcs_l):
                            nc.tensor.matmul(
                                pso[64 * j : 64 * j + 64, :qs],
                                V[:ks, kc, h * HD : (h + 1) * HD],
                                AT[:ks, h, kc, :qs],
                                start=(kc == 0),
                                stop=(kc == nkc - 1),
                                tile_position=(0, 64 * j),
                            )
                    nc.any.tensor_copy(OT[:, f, qo : qo + qs], pso[:, :qs])

                # head-mean of attention (tree sum in place) -> DRAM
                nc.vector.tensor_tensor(
                    E[:qs, 0:4, :kv_len], E[:qs, 0:4, :kv_len],
                    E[:qs, 4:8, :kv_len], ALU.add,
                )
                nc.vector.tensor_tensor(
                    E[:qs, 0:2, :kv_len], E[:qs, 0:2, :kv_len],
                    E[:qs, 2:4, :kv_len], ALU.add,
                )
                nc.vector.tensor_tensor(
                    E[:qs, 0, :kv_len], E[:qs, 0, :kv_len],
                    E[:qs, 1, :kv_len], ALU.add,
                )
                nc.sync.dma_start(attn_d[b, qo : qo + qs, :], E[:qs, 0, :kv_len])

            # ---- out-proj + residual + LN1 ----
            Y1 = apool.tile([P, 2, D], F32, tag="Y1")
            for qc, (qo, qs) in enumerate(qcs_l):
                psf = ps_proj.tile([P, 512], F32, tag="psp")
                for fc in range(NFC):
                    nc.tensor.matmul(
                        psf[:qs, :],
                        OT[:, fc, qo : qo + qs],
                        wo[:, fc, :],
                        start=(fc == 0),
                        stop=(bias_trivial and fc == NFC - 1),
                    )
                bias_row_mm(psf[:qs, :], f"{pre}_bo", qs)
                Y = ypool.tile([P, D], F32, tag="ytmp")
                nc.vector.tensor_tensor(
                    Y[:qs, :], psf[:qs, :], q_x[:qs, qc, :], ALU.add
                )
                ln_apply(Y[:qs, :], Y1[:qs, qc, :], qs,
                         f"{ffn}_g1r", f"{ffn}_b1r", None)

            # ---- Y1^T for FFN ----
            YT = apool.tile([P, NFC, 256], F32, tag="QT")
            for fc in range(NFC):
                tp = ps_tr.tile([P, 2, P], F32, tag="ps_tr")
                for qc, (qo, qs) in enumerate(qcs_l):
                    nc.tensor.transpose(
                        tp[:, qc, :qs],
                        Y1[:qs, qc, fc * P : (fc + 1) * P],
                        ident[:qs, :qs],
                    )
                    nc.any.tensor_copy(YT[:, fc, qo : qo + qs], tp[:, qc, :qs])

            # ---- FFN: H^T = W1^T Y1^T (+b1), leaky relu ----
            HT = apool.tile([P, NHC, 256], F32, tag="vht")
            for hc in range(NHC):
                psh = ps_proj.tile([P, 512], F32, tag="psp")
                for fc in range(NFC):
                    nc.tensor.matmul(
                        psh[:, :q_len],
                        w1[:, fc, hc * P : (hc + 1) * P],
                        YT[:, fc, :q_len],
                        start=(fc == 0),
                        stop=(bias_trivial and fc == NFC - 1),
                    )
                bias_row_mm_T(psh[:, :q_len], f"{ffn}_b1", q_len,
                              (hc * P, (hc + 1) * P))
                nc.vector.tensor_scalar_mul(HT[:, hc, :q_len], psh[:, :q_len], 0.01)
                nc.vector.tensor_tensor(
                    HT[:, hc, :q_len], HT[:, hc, :q_len], psh[:, :q_len], ALU.max
                )

            # ---- FFN2 + b2, residual, LN2 (+zero-masking), store ----
            for qc, (qo, qs) in enumerate(qcs_l):
                psz = ps_proj.tile([P, 512], F32, tag="psp")
                for hc in range(NHC):
                    nc.tensor.matmul(
                        psz[:qs, :],
                        HT[:, hc, qo : qo + qs],
                        w2[:, hc, :],
                        start=(hc == 0),
                        stop=(bias_trivial and hc == NHC - 1),
                    )
                bias_row_mm(psz[:qs, :], f"{ffn}_b2", qs)
                Z = ypool.tile([P, D], F32, tag="ytmp")
                nc.vector.tensor_tensor(
                    Z[:qs, :], psz[:qs, :], Y1[:qs, qc, :], ALU.add
                )
                OUTt = ypool.tile([P, D], F32, tag="ytmp")
                ln_apply(Z[:qs, :], OUTt[:qs, :], qs,
                         f"{ffn}_g2r", f"{ffn}_b2r", keep_c[:qs, qc : qc + 1])
                nc.sync.dma_start(out_d[b, qo : qo + qs, :], OUTt[:qs, :])

        # ================= batch loop =================
        for b in range(BPC):
            Xt = xpool.tile([P, 2, D], F32, tag="Xt")
            nc.sync.dma_start(Xt[:], tcr_x[b].rearrange("(c p) d -> p c d", p=P))
            Xp = xpool.tile([P, 2, D], F32, tag="Xp")
            nc.sync.dma_start(Xp[:, 0, :], pmhc_x[b, 0:P, :])
            nc.sync.dma_start(Xp[:64, 1, :], pmhc_x[b, P:LP, :])

            XTt = xpool.tile([P, NFC, LT], F32, tag="XTt")
            for fc in range(NFC):
                tp = ps_tr.tile([P, 2, P], F32, tag="ps_tr")
                for c, (co, cs) in enumerate(_chunks(LT)):
                    nc.tensor.transpose(
                        tp[:, c, :cs], Xt[:cs, c, fc * P : (fc + 1) * P],
                        ident[:cs, :cs],
                    )
                    nc.any.tensor_copy(XTt[:, fc, co : co + cs], tp[:, c, :cs])
            XTp = xpool.tile([P, NFC, LP], F32, tag="XTp")
            for fc in range(NFC):
                tp = ps_tr.tile([P, 2, P], F32, tag="ps_tr")
                for c, (co, cs) in enumerate(_chunks(LP)):
                    nc.tensor.transpose(
                        tp[:, c, :cs], Xp[:cs, c, fc * P : (fc + 1) * P],
                        ident[:cs, :cs],
                    )
                    nc.any.tensor_copy(XTp[:, fc, co : co + cs], tp[:, c, :cs])

            mrow_tp = mpool.tile([1, 256], F32, tag="mrow_tp")
            nc.sync.dma_start(mrow_tp[0:1, :LP], mask_tp[b][None, :])
            mrow_pt = mpool.tile([1, 256], F32, tag="mrow_pt")
            nc.sync.dma_start(mrow_pt[0:1, :LT], mask_pt[b][None, :])
            kc_tcr = mpool.tile([P, 2], F32, tag="kc_tcr")
            nc.sync.dma_start(kc_tcr[:], keep_tcr[b].rearrange("(c p) -> p c", p=P))
            kc_pep = mpool.tile([P, 2], F32, tag="kc_pep")
            nc.sync.dma_start(kc_pep[:, 0:1], keep_pep[b, 0:P][:, None])
            nc.sync.dma_start(kc_pep[:64, 1:2], keep_pep[b, P:LP][:, None])

            side("tp", "tcr", Xt, XTt, XTp, LT, LP,
                 mrow_tp[0:1, :LP], 0, kc_tcr, tcr_out, attn_tp, b)
            side("pt", "pep", Xp, XTp, XTt, LP, LT,
                 mrow_pt[0:1, :LT], 0, kc_pep, pep_out, attn_pt, b)

    nc.compile()
    return nc


_built = {}


def _get_nc(key):
    if key not in _built:
        _built[key] = _build(*key)
    return _built[key]


def _prep_maps(tcr_x, pmhc_x, peptide_mask, tcr_padding_mask, pmhc_padding_mask,
               params):
    p = {k: np.asarray(v) for k, v in params.items()}
    f32 = lambda a: np.ascontiguousarray(np.asarray(a), dtype=np.float32)

    ln_trivial = all(
        np.all(p[f"{s}_ln{i}_g"] == 1.0) and np.all(p[f"{s}_ln{i}_b"] == 0.0)
        for s in ("tcr", "pep")
        for i in (1, 2)
    )
    bias_trivial = all(
        np.all(p[k] == 0.0)
        for k in ("tp_bv", "tp_bo", "pt_bv", "pt_bo",
                  "tcr_b1", "tcr_b2", "pep_b1", "pep_b2")
    )

    peptide_mask = np.asarray(peptide_mask).astype(bool)
    tcr_padding_mask = np.asarray(tcr_padding_mask).astype(bool)
    pmhc_padding_mask = np.asarray(pmhc_padding_mask).astype(bool)

    mask_tp = np.where(peptide_mask, 0.0, NEG).astype(np.float32)
    mask_pt = np.where(tcr_padding_mask, NEG, 0.0).astype(np.float32)
    keep_tcr = (~tcr_padding_mask).astype(np.float32)
    keep_pep = (peptide_mask & ~pmhc_padding_mask).astype(np.float32)

    shared = {}
    for s in ("tp", "pt"):
        shared[f"{s}_wq"] = f32(p[f"{s}_Wq"]) * np.float32(0.125)
        shared[f"{s}_bq"] = f32(p[f"{s}_bq"]) * np.float32(0.125)
        shared[f"{s}_wk"] = f32(p[f"{s}_Wk"])
        shared[f"{s}_bk"] = f32(p[f"{s}_bk"])
        shared[f"{s}_wv"] = f32(p[f"{s}_Wv"])
        shared[f"{s}_wo"] = f32(p[f"{s}_Wo"]) * np.float32(8.0)
        if not bias_trivial:
            shared[f"{s}_bv"] = f32(p[f"{s}_bv"])
            shared[f"{s}_bo"] = f32(p[f"{s}_bo"])
    for s in ("tcr", "pep"):
        shared[f"{s}_w1"] = f32(p[f"{s}_W1"])
        shared[f"{s}_w2"] = f32(p[f"{s}_W2"])
        if not bias_trivial:
            shared[f"{s}_b1"] = f32(p[f"{s}_b1"])
            shared[f"{s}_b2"] = f32(p[f"{s}_b2"])
        if not ln_trivial:
            for i in (1, 2):
                shared[f"{s}_g{i}r"] = np.broadcast_to(
                    f32(p[f"{s}_ln{i}_g"]), (P, D)
                ).copy()
                shared[f"{s}_b{i}r"] = np.broadcast_to(
                    f32(p[f"{s}_ln{i}_b"]), (P, D)
                ).copy()

    tcr_x = f32(tcr_x)
    pmhc_x = f32(pmhc_x)
    in_maps = []
    for c in range(NCORES):
        sl = slice(BPC * c, BPC * (c + 1))
        m = dict(shared)
        m["tcr_x"] = tcr_x[sl]
        m["pmhc_x"] = pmhc_x[sl]
        m["mask_tp"] = np.ascontiguousarray(mask_tp[sl])
        m["mask_pt"] = np.ascontiguousarray(mask_pt[sl])
        m["keep_tcr"] = np.ascontiguousarray(keep_tcr[sl])
        m["keep_pep"] = np.ascontiguousarray(keep_pep[sl])
        in_maps.append(m)
    return (ln_trivial, bias_trivial), in_maps


_runners = {}


def _get_runner(key):
    """Build (once) a sharded jax jit over the 8 cores for this program."""
    if key in _runners:
        return _runners[key]
    import jax
    from concourse import bass2jax
    from jax.experimental.shard_map import shard_map
    from jax.sharding import Mesh, NamedSharding, PartitionSpec

    nc = _get_nc(key)
    bass2jax.install_neuronx_cc_hook()
    in_names, out_names, out_avals = [], [], []
    for alloc in nc.m.functions[0].allocations:
        if not isinstance(alloc, mybir.MemoryLocationSet):
            continue
        name = alloc.memorylocations[0].name
        if alloc.kind == "ExternalInput":
            in_names.append(name)
        elif alloc.kind == "ExternalOutput":
            out_names.append(name)
            out_avals.append(
                jax.core.ShapedArray(
                    tuple(alloc.tensor_shape), mybir.dt.np(alloc.dtype)
                )
            )
    partition_name = (
        nc.partition_id_tensor.name if nc.partition_id_tensor else None
    )
    if partition_name in in_names:
        in_names.remove(partition_name)
    n_params = len(in_names)
    all_in_names = list(in_names) + list(out_names)
    if partition_name:
        all_in_names.append(partition_name)

    def _body(*args):
        operands = list(args)
        if partition_name is not None:
            operands.append(bass2jax.partition_id_tensor())
        outs = bass2jax._bass_exec_p.bind(
            *operands,
            out_avals=tuple(out_avals),
            in_names=tuple(all_in_names),
            out_names=tuple(out_names),
            lowering_input_output_aliases=(),
            sim_require_finite=True,
            sim_require_nnan=True,
            nc=nc,
        )
        return tuple(outs)

    devices = jax.devices()[:NCORES]
    mesh = Mesh(np.asarray(devices), ("core",))
    n_outs = len(out_names)
    donate = tuple(range(n_params, n_params + n_outs))
    sharded = jax.jit(
        shard_map(
            _body,
            mesh=mesh,
            in_specs=(PartitionSpec("core"),) * (n_params + n_outs),
            out_specs=(PartitionSpec("core"),) * n_outs,
            check_rep=False,
        ),
        donate_argnums=donate,
        keep_unused=True,
    )
    r = {
        "sharded": sharded,
        "in_names": in_names,
        "out_names": out_names,
        "out_avals": out_avals,
        "sh": NamedSharding(mesh, PartitionSpec("core")),
    }
    _runners[key] = r
    return r


def _run_sharded(key, in_maps, timing_iters=0):
    """Run on 8 cores; optionally time `timing_iters` back-to-back execs.

    Returns (out_arrays_by_name_concat, per_iter_seconds_or_None).
    """
    import time as _time

    import jax

    r = _get_runner(key)
    concat_in = [
        np.concatenate([np.asarray(in_maps[c][nm]) for c in range(NCORES)], 0)
        for nm in r["in_names"]
    ]
    dev_in = [jax.device_put(a, r["sh"]) for a in concat_in]

    def zeros():
        return [
            jax.device_put(
                np.zeros((NCORES * av.shape[0], *av.shape[1:]), av.dtype),
                r["sh"],
            )
            for av in r["out_avals"]
        ]

    out = r["sharded"](*dev_in, *zeros())  # compile + warm
    jax.block_until_ready(out)
    per_iter = None
    if timing_iters:
        zsets = [zeros() for _ in range(timing_iters)]
        jax.block_until_ready(zsets)
        outs_t = []
        t0 = _time.perf_counter()
        for i in range(timing_iters):
            outs_t.append(r["sharded"](*dev_in, *zsets[i]))
        jax.block_until_ready(outs_t)
        per_iter = (_time.perf_counter() - t0) / timing_iters
        out = outs_t[-1]
    res = {}
    for i, nm in enumerate(r["out_names"]):
        av = r["out_avals"][i]
        res[nm] = np.asarray(out[i]).reshape(NCORES, *av.shape)
    return res, per_iter


def kernel(tcr_x, pmhc_x, peptide_mask, tcr_padding_mask, pmhc_padding_mask,
           params):
    key, in_maps = _prep_maps(
        tcr_x, pmhc_x, peptide_mask, tcr_padding_mask, pmhc_padding_mask, params
    )
    try:
        res, _ = _run_sharded(key, in_maps)
        outs = {nm: a.reshape(-1, *a.shape[2:]) for nm, a in res.items()}
        return (outs["tcr_out"], outs["pep_out"],
                outs["attn_tp"], outs["attn_pt"])
    except Exception:
        nc = _get_nc(key)
        rr = run_bass_kernel_spmd(nc, in_maps, list(range(NCORES)))
        outs = rr.results
        tcr_out = np.concatenate([outs[c]["tcr_out"] for c in range(NCORES)], 0)
        pep_out = np.concatenate([outs[c]["pep_out"] for c in range(NCORES)], 0)
        attn_tp = np.concatenate([outs[c]["attn_tp"] for c in range(NCORES)], 0)
        attn_pt = np.concatenate([outs[c]["attn_pt"] for c in range(NCORES)], 0)
        return tcr_out, pep_out, attn_tp, attn_pt
